# revision 1
# baseline (speedup 1.0000x reference)
"""GAT (2-layer, 8-head) forward on 8 Trainium2 NeuronCores.

Strategy (graph/data parallel, per the sharding hint):
 - Nodes sharded by destination across 8 cores (6250 each, padded to 6272 =
   49 tiles of 128). Edges live on the core owning their dst.
 - Per layer, each core computes per-node records [hp(bf16) | a_src(f32)]
   for its own nodes; the record table is replicated with AllGathers. The
   node range is split into Q tile-aligned quarters with one AllGather per
   quarter, and edges are classified by source quarter, so gathers of
   early quarters overlap later collectives.
 - Edge phase: edges grouped into 128-edge blocks per dst tile (padded
   uniformly across cores so all 8 cores run the same program). Per chunk,
   dma_gather pulls source records; host-precomputed one-hot indicator
   matrices (Ind and its transpose) are streamed in bf16. Per block:
   a_dst is expanded via an IndT matmul against the local a_dst tile,
   attention logits -> LeakyReLU (max form) -> exp with broadcast to the
   16 channels per head on the ACT engine, messages = exp * hp in bf16,
   and PSUM-accumulated indicator matmuls produce both the weighted
   message sums and the softmax denominators (segment softmax division is
   deferred to node level). No max-subtraction is needed: logits are
   O(0.3).
 - Readout: partial per-graph sums via matmul with a host-built graph
   indicator, AllReduce, then the output projection on-chip.
"""
import os
import sys
sys.path.insert(0, "/opt/trn_rl_repo")

import numpy as np
import ml_dtypes

import concourse.bass as bass
import concourse.bacc as bacc
import concourse.mybir as mybir
from concourse.masks import make_identity
from concourse.tile import TileContext
from concourse.bass_utils import run_bass_kernel_spmd

f32 = mybir.dt.float32
bf16 = mybir.dt.bfloat16
i16 = mybir.dt.int16

NC = 8
P = 128
H = 8
CH = 16
NEG_SLOPE = 0.2
TPC = 2          # dst tiles per gather chunk
QN = 2           # source halves (per-half AllGather; keeps idx in int16)
REC = 256        # record row elems (bf16): [hp(128) | a_src f32 as 16 | pad]

_PLAN_CACHE = {}
TRACE = False
LAST_EXEC_NS = None
LAST_RESULTS = None


def _ceil_to(x, m):
    return (x + m - 1) // m * m


def _wrap16(idx):
    """Logical idx list -> [128, n/16] int16 (16-wrap, replicated x8)."""
    n = idx.shape[0]
    out = np.zeros((16, max(n // 16, 1)), dtype=np.int16)
    if n:
        out[np.arange(n) % 16, np.arange(n) // 16] = idx
    return np.tile(out, (8, 1))


def _plan(N, E, D, G, L, edge_index, batch):
    """Host-side partition of the graph; uniform structure across cores."""
    NPCR = N // NC
    NPC = _ceil_to(NPCR, P)
    T = NPC // P
    Q = min(QN, T)

    # tile-aligned quarter split
    base, rem = divmod(T, Q)
    qtiles = [base + (1 if q < rem else 0) for q in range(Q)]
    qt0 = np.cumsum([0] + qtiles)           # tile start per quarter
    qrow0 = qt0 * P                          # local row start per quarter
    qsize = [qtiles[q] * P for q in range(Q)]

    src = np.asarray(edge_index[0], dtype=np.int64)
    dst = np.asarray(edge_index[1], dtype=np.int64)
    core = np.minimum(dst // NPCR, NC - 1)
    dstl = dst - core * NPCR
    tile = dstl // P
    s_core = np.minimum(src // NPCR, NC - 1)
    s_local = src - s_core * NPCR           # local row on source core
    s_q = np.searchsorted(qrow0[1:], s_local, side="right")
    s_row = s_core * np.asarray(qsize)[s_q] + (s_local - qrow0[s_q])

    lists = [[[None] * Q for _ in range(T)] for _ in range(NC)]
    order = np.lexsort((s_row, s_q, tile, core))
    co, to, qo = core[order], tile[order], s_q[order]
    for c in range(NC):
        cm = co == c
        for t in range(T):
            tm = cm & (to == t)
            for q in range(Q):
                lists[c][t][q] = order[tm & (qo == q)]

    BQ = np.zeros((T, Q), dtype=np.int64)
    for t in range(T):
        for q in range(Q):
            BQ[t, q] = max(-(-len(lists[c][t][q]) // P) for c in range(NC))
    NBT = BQ.sum(axis=1)
    NBLK = int(NBT.sum())
    NQB = [int(BQ[:, q].sum()) for q in range(Q)]   # blocks per class

    chunks = []   # (tiles, per-class block counts, total blocks)
    for t0 in range(0, T, TPC):
        ts = list(range(t0, min(t0 + TPC, T)))
        kq = [int(BQ[ts, q].sum()) for q in range(Q)]
        chunks.append((ts, kq, int(NBT[ts].sum())))

    per_core = []
    dst_in_tile = (dstl % P).astype(np.int64)
    for c in range(NC):
        ivals = [np.zeros(NQB[q] * P, dtype=np.int16) for q in range(Q)]
        pq = [0] * Q
        dcol = np.full((P, NBLK), P, dtype=np.int64)
        blk = 0
        for t in range(T):
            for q in range(Q):
                el = lists[c][t][q]
                nb = int(BQ[t, q])
                nslots = nb * P
                rr = np.zeros(nslots, dtype=np.int16)
                rr[: len(el)] = s_row[el].astype(np.int16)
                cc = np.full(nslots, P, dtype=np.int64)
                cc[: len(el)] = dst_in_tile[el]
                ivals[q][pq[q]: pq[q] + nslots] = rr
                pq[q] += nslots
                dcol[:, blk: blk + nb] = cc.reshape(nb, P).T
                blk += nb
        eye = np.eye(P + 1, P, dtype=ml_dtypes.bfloat16)
        ind_s = eye[dcol]                       # [P(lane), NBLK, P(seg)]
        indT_s = np.ascontiguousarray(ind_s.transpose(2, 1, 0))
        per_core.append(dict(
            idx=[_wrap16(ivals[q]) for q in range(Q)],
            ind_s=np.ascontiguousarray(ind_s.reshape(P, NBLK * P)),
            indT_s=indT_s.reshape(P, NBLK * P),
        ))

    b = np.asarray(batch, dtype=np.int64)
    cnt = np.bincount(b, minlength=G).astype(np.float32)
    invcnt = (1.0 / np.maximum(cnt, 1.0)).reshape(G, 1).astype(np.float32)
    for c in range(NC):
        gind = np.zeros((P, T, G), dtype=np.float32)
        for t in range(T):
            n0 = c * NPCR + t * P
            n1 = min(n0 + P, (c + 1) * NPCR)
            for p in range(n1 - n0):
                gind[p, t, b[n0 + p]] = 1.0
        per_core[c]["gind"] = gind

    return dict(NPCR=NPCR, NPC=NPC, T=T, Q=Q, qt0=qt0, qrow0=qrow0,
                qsize=qsize, NBT=NBT, BQ=BQ, NBLK=NBLK, NQB=NQB,
                chunks=chunks, per_core=per_core, invcnt=invcnt)


def _build(plan, D, G, L, NUM_CLASSES):
    T, NBLK, Q = plan["T"], plan["NBLK"], plan["Q"]
    NPC = plan["NPC"]
    BQ, NQB, NBT = plan["BQ"], plan["NQB"], plan["NBT"]
    qt0, qrow0, qsize = plan["qt0"], plan["qrow0"], plan["qsize"]
    chunks = plan["chunks"]

    nc = bacc.Bacc("TRN2", target_bir_lowering=False, debug=False,
                   num_devices=NC)

    def din(name, shape, dt=f32):
        return nc.declare_dram_parameter(name, list(shape), dt, isOutput=False)

    xT = din("xT", [P, NPC])
    in_w = din("in_w", [P, D])
    inb_rep = din("inb_rep", [P, D])
    conv_w = din("conv_w", [L, P, D], bf16)
    wa_src = din("wa_src", [L, P, H], bf16)
    wa_dst = din("wa_dst", [L, P, H], bf16)
    convb_rep = din("convb_rep", [L, P, D])
    out_w = din("out_w", [P, NUM_CLASSES])
    outb_rep = din("outb_rep", [G, NUM_CLASSES])
    invcnt = din("invcnt", [G, 1])
    idx_in = [din(f"idx{q}", [P, max(NQB[q] * 8, 1)], i16) for q in range(Q)]
    ind_s = din("ind_s", [P, NBLK * P], bf16)
    indT_s = din("indT_s", [P, NBLK * P], bf16)
    gind = din("gind", [P, T * G])

    out = nc.declare_dram_parameter("out", [G, NUM_CLASSES], f32,
                                    isOutput=True)

    with TileContext(nc) as tc:
        with (
            tc.tile_pool(name="const", bufs=1) as cp,
            tc.tile_pool(name="sbuf", bufs=3) as sb,
            tc.tile_pool(name="gath", bufs=2) as gp,
            tc.tile_pool(name="psum", bufs=3, space="PSUM") as ps,
            tc.tile_pool(name="psum_small", bufs=3, space="PSUM") as pss,
            tc.tile_pool(name="psum_adb", bufs=2, space="PSUM") as psa,
            tc.tile_pool(name="dram", bufs=1, space="DRAM") as dr,
        ):
            ident = cp.tile([P, P], bf16)
            make_identity(nc, ident[:])
            in_w_t = cp.tile([P, D], f32)
            nc.sync.dma_start(in_w_t[:], in_w[:])
            inb_t = cp.tile([P, D], f32)
            nc.sync.dma_start(inb_t[:], inb_rep[:])
            cw_t = cp.tile([P, L, D], bf16)
            nc.sync.dma_start(cw_t[:], conv_w[:].transpose([1, 0, 2]))
            was_t = cp.tile([P, L, H], bf16)
            nc.sync.dma_start(was_t[:], wa_src[:].transpose([1, 0, 2]))
            wad_t = cp.tile([P, L, H], bf16)
            nc.sync.dma_start(wad_t[:], wa_dst[:].transpose([1, 0, 2]))
            cb_t = cp.tile([P, L, D], f32)
            nc.sync.dma_start(cb_t[:], convb_rep[:].transpose([1, 0, 2]))
            ow_t = cp.tile([P, NUM_CLASSES], f32)
            nc.sync.dma_start(ow_t[:], out_w[:])
            ob_t = cp.tile([G, NUM_CLASSES], f32)
            nc.sync.dma_start(ob_t[:], outb_rep[:])
            ic_t = cp.tile([G, 1], f32)
            nc.sync.dma_start(ic_t[:], invcnt[:])
            idx_t = []
            for q in range(Q):
                it = cp.tile([P, max(NQB[q] * 8, 1)], i16, tag=f"idx{q}")
                nc.sync.dma_start(it[:], idx_in[q][:])
                idx_t.append(it)
            gind_t = cp.tile([P, T, G], f32)
            nc.sync.dma_start(gind_t[:], gind[:].rearrange("p (t g) -> p t g", t=T))

            h_own = cp.tile([P, T, D], f32)
            rec_sb = cp.tile([P, T, REC], bf16)
            adl_bf = cp.tile([P, T, H], bf16)

            # ---- in_proj ----
            for t in range(T):
                xt = sb.tile([P, P], f32, tag="xt")
                nc.sync.dma_start(xt[:], xT[:, t * P:(t + 1) * P])
                h0 = ps.tile([P, D], f32, space="PSUM", tag="big")
                nc.tensor.matmul(out=h0[:], lhsT=xt[:], rhs=in_w_t[:],
                                 start=True, stop=True)
                nc.vector.tensor_tensor(out=h_own[:, t, :], in0=h0[:],
                                        in1=inb_t[:], op=mybir.AluOpType.add)

            for layer in range(L):
                # ---- node phase ----
                for t in range(T):
                    hb = sb.tile([P, D], bf16, tag="hb")
                    nc.vector.tensor_copy(hb[:], h_own[:, t, :])
                    hT_ps = ps.tile([P, P], bf16, space="PSUM", tag="big")
                    nc.tensor.transpose(out=hT_ps[:], in_=hb[:], identity=ident[:])
                    hTb = sb.tile([P, P], bf16, tag="hTb")
                    nc.vector.tensor_copy(hTb[:], hT_ps[:])
                    hp_ps = ps.tile([P, D], f32, space="PSUM", tag="big")
                    nc.tensor.matmul(out=hp_ps[:], lhsT=hTb[:],
                                     rhs=cw_t[:, layer, :], start=True, stop=True)
                    as_ps = pss.tile([P, H], f32, space="PSUM", tag="small")
                    nc.tensor.matmul(out=as_ps[:], lhsT=hTb[:],
                                     rhs=was_t[:, layer, :], start=True, stop=True)
                    ad_ps = pss.tile([P, H], f32, space="PSUM", tag="small")
                    nc.tensor.matmul(out=ad_ps[:], lhsT=hTb[:],
                                     rhs=wad_t[:, layer, :], start=True, stop=True)
                    nc.vector.tensor_copy(rec_sb[:, t, 0:D], hp_ps[:])
                    nc.vector.tensor_copy(
                        rec_sb[:, t, D:D + 2 * H].bitcast(f32), as_ps[:])
                    nc.vector.tensor_copy(adl_bf[:, t, :], ad_ps[:])

                # records to DRAM + per-quarter AllGather
                r_own = dr.tile([NPC, REC], bf16, tag=f"r_own{layer}")
                r_all = []
                for q in range(Q):
                    t0, t1 = int(qt0[q]), int(qt0[q + 1])
                    nc.sync.dma_start(
                        r_own[:].rearrange("(t p) r -> p t r", p=P)[:, t0:t1, :],
                        rec_sb[:, t0:t1, :])
                    rq = dr.tile([NC * qsize[q], REC], bf16,
                                 tag=f"r_all{layer}_{q}", addr_space="Shared")
                    nc.gpsimd.collective_compute(
                        "AllGather", mybir.AluOpType.bypass,
                        replica_groups=[list(range(NC))],
                        ins=[r_own[int(qrow0[q]):int(qrow0[q]) + qsize[q], :].opt()],
                        outs=[rq.opt()],
                    )
                    r_all.append(rq)

                # ---- edge phase ----
                q_off = [0] * Q      # per-class block offset (global)
                ad_off = 0           # all-class block offset (global)
                for (ctiles, kq, nbt_c) in chunks:
                    gq = [None] * Q
                    for q in range(Q):
                        if kq[q] == 0:
                            continue
                        gtile = gp.tile([P, kq[q], REC], bf16, tag=f"g{q}")
                        gq[q] = gtile
                        nc.gpsimd.dma_gather(
                            out_ap=gq[q][:], in_ap=r_all[q][:],
                            idxs_ap=idx_t[q][:, q_off[q] * 8:(q_off[q] + kq[q]) * 8],
                            num_idxs=kq[q] * P, num_idxs_reg=kq[q] * P,
                            elem_size=REC, single_packet=False)
                    istr = gp.tile([P, nbt_c, P], bf16, tag="istr")
                    nc.sync.dma_start(
                        istr[:], ind_s[:, ad_off * P:(ad_off + nbt_c) * P]
                        .rearrange("p (b s) -> p b s", s=P))
                    itstr = gp.tile([P, nbt_c, P], bf16, tag="itstr")
                    nc.sync.dma_start(
                        itstr[:], indT_s[:, ad_off * P:(ad_off + nbt_c) * P]
                        .rearrange("p (b s) -> p b s", s=P))

                    qi = [0] * Q     # per-class cursor within gq tiles
                    ci = 0           # block cursor within chunk
                    for t in ctiles:
                        nb = int(NBT[t])
                        S_ps = ps.tile([P, D], f32, space="PSUM", tag="big")
                        D_ps = pss.tile([P, H], f32, space="PSUM", tag="small")

                        # a_dst expansion via IndT matmuls into one psum
                        adb_ps = psa.tile([P, nb * H], f32, space="PSUM",
                                          tag="adb")
                        for j in range(nb):
                            nc.tensor.matmul(
                                out=adb_ps[:, j * H:(j + 1) * H],
                                lhsT=itstr[:, ci + j, :], rhs=adl_bf[:, t, :],
                                start=True, stop=True)

                        # batched attention -> leaky -> exp-expand
                        att_t = sb.tile([P, nb * H], f32, tag="att_t")
                        a0 = 0
                        for q in range(Q):
                            k = int(BQ[t, q])
                            if k == 0:
                                continue
                            nc.vector.tensor_tensor(
                                out=att_t[:, a0 * H:(a0 + k) * H]
                                    .rearrange("p (k h) -> p k h", k=k),
                                in0=gq[q][:, qi[q]:qi[q] + k, D:D + 2 * H]
                                    .bitcast(f32),
                                in1=adb_ps[:, a0 * H:(a0 + k) * H]
                                    .rearrange("p (k h) -> p k h", k=k),
                                op=mybir.AluOpType.add)
                            a0 += k
                        lk_t = sb.tile([P, nb * H], f32, tag="lk_t")
                        nc.vector.scalar_tensor_tensor(
                            out=lk_t[:], in0=att_t[:], scalar=NEG_SLOPE,
                            in1=att_t[:], op0=mybir.AluOpType.mult,
                            op1=mybir.AluOpType.max)
                        exf_t = sb.tile([P, nb * P], bf16, tag="exf_t")
                        nc.scalar.activation(
                            out=exf_t[:].rearrange("p (b h c) -> p b h c",
                                                   b=nb, h=H),
                            in_=lk_t[:].rearrange("p (b h) -> p b h", b=nb)
                                .unsqueeze(3).broadcast_to([P, nb, H, CH]),
                            func=mybir.ActivationFunctionType.Exp)

                        # messages (paired) + indicator matmuls
                        j = 0
                        for q in range(Q):
                            kq_t = int(BQ[t, q])
                            jq = 0
                            while jq < kq_t:
                                pair = min(2, kq_t - jq)
                                msg = sb.tile([P, 2 * P], bf16, tag="msg")
                                nc.vector.tensor_tensor(
                                    out=msg[:, :pair * P]
                                        .rearrange("p (b d) -> p b d", b=pair),
                                    in0=gq[q][:, qi[q] + jq:qi[q] + jq + pair,
                                              0:D],
                                    in1=exf_t[:, j * P:(j + pair) * P]
                                        .rearrange("p (b d) -> p b d", b=pair),
                                    op=mybir.AluOpType.mult)
                                for b in range(pair):
                                    first = (j == 0)
                                    last = (j == nb - 1)
                                    nc.tensor.matmul(
                                        out=S_ps[:], lhsT=istr[:, ci + j, :],
                                        rhs=msg[:, b * P:(b + 1) * P],
                                        start=first, stop=last)
                                    nc.tensor.matmul(
                                        out=D_ps[:], lhsT=istr[:, ci + j, :],
                                        rhs=exf_t[:, j * P:(j + 1) * P]
                                            .rearrange("p (h c) -> p h c",
                                                       h=H)[:, :, 0],
                                        start=first, stop=last)
                                    j += 1
                                jq += pair
                            qi[q] += kq_t
                        ci += nb

                        # epilogue: divide, bias, ELU
                        d8 = sb.tile([P, H], f32, tag="d8")
                        nc.vector.tensor_scalar(
                            out=d8[:], in0=D_ps[:], scalar1=1e-12, scalar2=None,
                            op0=mybir.AluOpType.max)
                        rc = sb.tile([P, H], f32, tag="rc")
                        nc.vector.reciprocal(rc[:], d8[:])
                        hr = sb.tile([P, D], f32, tag="hr")
                        nc.vector.tensor_tensor(
                            out=hr[:].rearrange("p (h c) -> p h c", h=H),
                            in0=S_ps[:].rearrange("p (h c) -> p h c", h=H),
                            in1=rc[:].unsqueeze(2).broadcast_to([P, H, CH]),
                            op=mybir.AluOpType.mult)
                        h1 = sb.tile([P, D], f32, tag="h1")
                        nc.vector.tensor_tensor(
                            out=h1[:], in0=hr[:], in1=cb_t[:, layer, :],
                            op=mybir.AluOpType.add)
                        ng = sb.tile([P, D], f32, tag="ng")
                        nc.vector.tensor_scalar(
                            out=ng[:], in0=h1[:], scalar1=0.0, scalar2=None,
                            op0=mybir.AluOpType.min)
                        pz = sb.tile([P, D], f32, tag="pz")
                        nc.scalar.activation(
                            out=pz[:], in_=h1[:],
                            func=mybir.ActivationFunctionType.Relu)
                        em = sb.tile([P, D], f32, tag="em")
                        nc.scalar.activation(
                            out=em[:], in_=ng[:],
                            func=mybir.ActivationFunctionType.Exp)
                        nc.vector.scalar_tensor_tensor(
                            out=h_own[:, t, :], in0=em[:], scalar=-1.0,
                            in1=pz[:], op0=mybir.AluOpType.add,
                            op1=mybir.AluOpType.add)
                    for q in range(Q):
                        q_off[q] += kq[q]
                    ad_off += nbt_c

            # ---- readout ----
            pool_ps = ps.tile([P, G], f32, space="PSUM", tag="big")
            for t in range(T):
                nc.tensor.matmul(out=pool_ps[:], lhsT=h_own[:, t, :],
                                 rhs=gind_t[:, t, :],
                                 start=(t == 0), stop=(t == T - 1))
            pool_sb = sb.tile([P, G], f32, tag="pool_sb")
            nc.vector.tensor_copy(pool_sb[:], pool_ps[:])
            ar_in = dr.tile([P, G], f32, tag="ar_in")
            nc.sync.dma_start(ar_in[:], pool_sb[:])
            ar_out = dr.tile([P, G], f32, tag="ar_out", addr_space="Shared")
            nc.gpsimd.collective_compute(
                "AllReduce", mybir.AluOpType.add,
                replica_groups=[list(range(NC))],
                ins=[ar_in.opt()], outs=[ar_out.opt()],
            )
            arT = sb.tile([P, G], f32, tag="arT")
            nc.sync.dma_start(arT[:], ar_out[:])
            log_ps = pss.tile([G, NUM_CLASSES], f32, space="PSUM", tag="small")
            nc.tensor.matmul(out=log_ps[:], lhsT=arT[:], rhs=ow_t[:],
                             start=True, stop=True)
            sc = sb.tile([G, NUM_CLASSES], f32, tag="sc")
            nc.vector.tensor_scalar(
                out=sc[:], in0=log_ps[:], scalar1=ic_t[:, :1], scalar2=None,
                op0=mybir.AluOpType.mult)
            ofin = sb.tile([G, NUM_CLASSES], f32, tag="ofin")
            nc.vector.tensor_tensor(out=ofin[:], in0=sc[:], in1=ob_t[:],
                                    op=mybir.AluOpType.add)
            nc.sync.dma_start(out[:], ofin[:])

    nc.finalize()
    return nc


def kernel(x, edge_index, batch, in_w, in_b, conv_w, conv_b, att_src, att_dst,
           out_w, out_b):
    x = np.asarray(x, dtype=np.float32)
    edge_index = np.asarray(edge_index)
    batch = np.asarray(batch)
    in_w = np.asarray(in_w, dtype=np.float32)
    in_b = np.asarray(in_b, dtype=np.float32)
    conv_w = np.asarray(conv_w, dtype=np.float32)
    conv_b = np.asarray(conv_b, dtype=np.float32)
    att_src = np.asarray(att_src, dtype=np.float32)
    att_dst = np.asarray(att_dst, dtype=np.float32)
    out_w = np.asarray(out_w, dtype=np.float32)
    out_b = np.asarray(out_b, dtype=np.float32)

    N, F_IN = x.shape
    E = edge_index.shape[1]
    L, D, _ = conv_w.shape
    G = int(batch.max()) + 1
    NUM_CLASSES = out_w.shape[1]

    key = (N, E, F_IN, D, G, L,
           hash(edge_index.tobytes()), hash(batch.tobytes()))
    if key not in _PLAN_CACHE:
        plan = _plan(N, E, D, G, L, edge_index, batch)
        nc = _build(plan, D, G, L, NUM_CLASSES)
        _PLAN_CACHE[key] = (plan, nc)
    plan, nc = _PLAN_CACHE[key]

    NPCR, NPC, T, Q = plan["NPCR"], plan["NPC"], plan["T"], plan["Q"]

    A_src = np.zeros((L, D, H), dtype=np.float32)
    A_dst = np.zeros((L, D, H), dtype=np.float32)
    for l in range(L):
        for h in range(H):
            A_src[l, h * CH:(h + 1) * CH, h] = att_src[l, h]
            A_dst[l, h * CH:(h + 1) * CH, h] = att_dst[l, h]
    wa_src = np.einsum("lfd,ldh->lfh", conv_w, A_src).astype(ml_dtypes.bfloat16)
    wa_dst = np.einsum("lfd,ldh->lfh", conv_w, A_dst).astype(ml_dtypes.bfloat16)
    conv_w_bf = conv_w.astype(ml_dtypes.bfloat16)
    inb_rep = np.tile(in_b, (P, 1)).astype(np.float32)
    convb_rep = np.tile(conv_b[:, None, :], (1, P, 1)).astype(np.float32)
    outb_rep = np.tile(out_b, (G, 1)).astype(np.float32)

    in_maps = []
    for c in range(NC):
        xs = np.zeros((NPC, F_IN), dtype=np.float32)
        xs[:NPCR] = x[c * NPCR:(c + 1) * NPCR]
        pc = plan["per_core"][c]
        m = dict(
            xT=np.ascontiguousarray(xs.T),
            in_w=in_w, inb_rep=inb_rep,
            conv_w=conv_w_bf, wa_src=wa_src, wa_dst=wa_dst,
            convb_rep=convb_rep, out_w=out_w, outb_rep=outb_rep,
            invcnt=plan["invcnt"],
            ind_s=pc["ind_s"], indT_s=pc["indT_s"],
            gind=np.ascontiguousarray(pc["gind"].reshape(P, T * G)),
        )
        for q in range(Q):
            m[f"idx{q}"] = pc["idx"][q]
        in_maps.append(m)

    global LAST_EXEC_NS, LAST_RESULTS
    res = run_bass_kernel_spmd(nc, in_maps, list(range(NC)), trace=TRACE)
    LAST_EXEC_NS = res.exec_time_ns
    LAST_RESULTS = res
    return np.asarray(res.results[0]["out"], dtype=np.float32)



# revision 11
# speedup vs baseline: 1.8294x; 1.8294x over previous
"""GAT (2-layer, 8-head) forward on 8 Trainium2 NeuronCores.

Strategy (graph/data parallel):
 - Nodes sharded by destination across 8 cores (6250 each, padded to 6272 =
   49 tiles of 128). Edges live on the core owning their dst.
 - Layer 0 is fully replicated: in_proj and the layer-0 GATConv projection
   fold into one host-precomputed matrix W0cat = [in_w@conv_w0 | ..@A_src |
   ..@A_dst], so every core computes the full-graph record table
   [hp | a_src | a_dst] (bf16) with one matmul per 128-node tile and writes
   it to local DRAM — no collective for layer 0 at all.
 - Layer 1 records are produced tile-by-tile inside the layer-0 edge-phase
   epilogue (transpose + one 144-col matmul) and shipped with a SINGLE
   AllGather (the cost model strongly rewards one large transfer over
   several small ones).
 - Edge phase: edges grouped into 128-edge blocks per dst tile, classified
   into 2 classes by source core half so gather indices stay int16 against
   a 25088-row window of the record table. Per chunk (2 dst tiles):
   dma_gather pulls source records (512B each); the one-hot indicator Ind
   is generated ON-CHIP (DVE is_equal against an iota row, from a tiny
   dcol table); its transpose IndT (needed as matmul lhsT for the a_dst
   expansion) is streamed in bf16. Attention: logits -> LeakyReLU -> exp
   kept at [128, nb*H] (no per-channel expansion; the message multiply
   broadcasts exp over the 16 channels/head), messages = exp * hp in bf16,
   PSUM-accumulated indicator matmuls produce weighted message sums and
   softmax denominators. Segment softmax division deferred to node level.
 - Readout: partial per-graph sums via bf16 matmul with a host-built graph
   indicator, AllGather + on-chip sum (cheaper than AllReduce in the cost
   model), then the output projection.
"""
import os
import sys
sys.path.insert(0, "/opt/trn_rl_repo")

import numpy as np
import ml_dtypes

import concourse.bass as bass
import concourse.bacc as bacc
import concourse.mybir as mybir
from concourse.masks import make_identity
from concourse.tile import TileContext
from concourse.bass_utils import run_bass_kernel_spmd

f32 = mybir.dt.float32
bf16 = mybir.dt.bfloat16
i16 = mybir.dt.int16

NC = 8
P = 128
H = 8
CH = 16
NEG_SLOPE = 0.2
TPC = 2          # dst tiles per gather chunk
QN = 2           # source classes (by core half; keeps gather idx in int16)
REC = 256        # record row elems (bf16): [hp(128) | a_src(8) | a_dst(8) | pad]
RW = 144         # populated record elems

_PLAN_CACHE = {}
TRACE = False
LAST_EXEC_NS = None
LAST_RESULTS = None


def _ceil_to(x, m):
    return (x + m - 1) // m * m


def _wrap16(idx):
    """Logical idx list -> [128, n/16] int16 (16-wrap, replicated x8)."""
    n = idx.shape[0]
    out = np.zeros((16, max(n // 16, 1)), dtype=np.int16)
    if n:
        out[np.arange(n) % 16, np.arange(n) // 16] = idx
    return np.tile(out, (8, 1))


def _plan(N, E, D, G, L, edge_index, batch):
    """Host-side partition of the graph; uniform structure across cores."""
    NPCR = N // NC
    NPC = _ceil_to(NPCR, P)
    T = NPC // P
    HALF = NC // QN

    src = np.asarray(edge_index[0], dtype=np.int64)
    dst = np.asarray(edge_index[1], dtype=np.int64)
    core = np.minimum(dst // NPCR, NC - 1)
    dstl = dst - core * NPCR
    tile = dstl // P
    s_core = np.minimum(src // NPCR, NC - 1)
    s_local = src - s_core * NPCR
    s_q = s_core // HALF
    s_row = (s_core % HALF) * NPC + s_local      # idx within class window

    lists = [[[None] * QN for _ in range(T)] for _ in range(NC)]
    order = np.lexsort((s_row, s_q, tile, core))
    co, to, qo = core[order], tile[order], s_q[order]
    for c in range(NC):
        cm = co == c
        for t in range(T):
            tm = cm & (to == t)
            for q in range(QN):
                lists[c][t][q] = order[tm & (qo == q)]

    BQ = np.zeros((T, QN), dtype=np.int64)
    for t in range(T):
        for q in range(QN):
            BQ[t, q] = max(-(-len(lists[c][t][q]) // P) for c in range(NC))
    NBT = BQ.sum(axis=1)
    NBLK = int(NBT.sum())
    NQB = [int(BQ[:, q].sum()) for q in range(QN)]

    chunks = []   # (tiles, per-class block counts, total blocks)
    for t0 in range(0, T, TPC):
        ts = list(range(t0, min(t0 + TPC, T)))
        kq = [int(BQ[ts, q].sum()) for q in range(QN)]
        chunks.append((ts, kq, int(NBT[ts].sum())))

    per_core = []
    dst_in_tile = (dstl % P).astype(np.int64)
    for c in range(NC):
        ivals = [np.zeros(NQB[q] * P, dtype=np.int16) for q in range(QN)]
        pq = [0] * QN
        dcol = np.full((P, NBLK), P, dtype=np.int64)
        blk = 0
        for t in range(T):
            for q in range(QN):
                el = lists[c][t][q]
                nb = int(BQ[t, q])
                nslots = nb * P
                rr = np.zeros(nslots, dtype=np.int16)
                rr[: len(el)] = s_row[el].astype(np.int16)
                cc = np.full(nslots, P, dtype=np.int64)
                cc[: len(el)] = dst_in_tile[el]
                ivals[q][pq[q]: pq[q] + nslots] = rr
                pq[q] += nslots
                dcol[:, blk: blk + nb] = cc.reshape(nb, P).T
                blk += nb
        eye = np.eye(P + 1, P, dtype=ml_dtypes.bfloat16)
        ind_s = eye[dcol]                       # [P(lane), NBLK, P(seg)]
        indT_s = np.ascontiguousarray(ind_s.transpose(2, 1, 0))
        per_core.append(dict(
            idx=[_wrap16(ivals[q]) for q in range(QN)],
            indT_s=indT_s.reshape(P, NBLK * P),
            dcol=dcol.astype(np.float32),
        ))

    b = np.asarray(batch, dtype=np.int64)
    cnt = np.bincount(b, minlength=G).astype(np.float32)
    invcnt = (1.0 / np.maximum(cnt, 1.0)).reshape(G, 1).astype(np.float32)
    for c in range(NC):
        gind = np.zeros((P, T, G), dtype=np.float32)
        for t in range(T):
            n0 = c * NPCR + t * P
            n1 = min(n0 + P, (c + 1) * NPCR)
            for p in range(n1 - n0):
                gind[p, t, b[n0 + p]] = 1.0
        per_core[c]["gind"] = gind.astype(ml_dtypes.bfloat16)

    return dict(NPCR=NPCR, NPC=NPC, T=T, NBT=NBT, BQ=BQ, NBLK=NBLK, NQB=NQB,
                chunks=chunks, per_core=per_core, invcnt=invcnt)


def _build(plan, D, G, L, NUM_CLASSES):
    T, NBLK = plan["T"], plan["NBLK"]
    NPC = plan["NPC"]
    BQ, NQB, NBT = plan["BQ"], plan["NQB"], plan["NBT"]
    chunks = plan["chunks"]
    GT = NC * T               # global tiles (layer-0 replicated pass)
    CW = NPC * (NC // QN)     # gather class window rows

    nc = bacc.Bacc("TRN2", target_bir_lowering=False, debug=False,
                   num_devices=NC)

    def din(name, shape, dt=f32):
        return nc.declare_dram_parameter(name, list(shape), dt, isOutput=False)

    NBMAX = max(c[2] for c in chunks)

    xT_all = din("xT_all", [P, NC * NPC], bf16)
    xT_own = din("xT_own", [P, NPC], bf16)
    w0cat = din("w0cat", [P, RW], bf16)
    b0cat = din("b0cat", [P, RW])
    w1cat = din("w1cat", [P, RW], bf16)
    convb_rep = din("convb_rep", [L, P, D])
    out_w = din("out_w", [P, NUM_CLASSES])
    outb_rep = din("outb_rep", [G, NUM_CLASSES])
    invcnt = din("invcnt", [G, 1])
    iota3_f = din("iota3_f", [P, P * NBMAX], bf16)
    dcol_in = din("dcol", [P, NBLK], bf16)
    idx_in = [din(f"idx{q}", [P, max(NQB[q] * 8, 1)], i16) for q in range(QN)]
    indT_s = din("indT_s", [P, NBLK * P], bf16)
    gind = din("gind", [P, T * G], bf16)

    out = nc.declare_dram_parameter("out", [G, NUM_CLASSES], f32,
                                    isOutput=True)

    with TileContext(nc) as tc:
        with (
            tc.tile_pool(name="const", bufs=1) as cp,
            tc.tile_pool(name="sbuf", bufs=3) as sb,
            tc.tile_pool(name="gath", bufs=2) as gp,
            tc.tile_pool(name="psum", bufs=3, space="PSUM") as ps,
            tc.tile_pool(name="psum_small", bufs=2, space="PSUM") as pss,
            tc.tile_pool(name="psum_adb", bufs=2, space="PSUM") as psa,
            tc.tile_pool(name="dram", bufs=1, space="DRAM") as dr,
        ):
            ident = cp.tile([P, P], bf16)
            make_identity(nc, ident[:])
            w0_t = cp.tile([P, RW], bf16)
            nc.sync.dma_start(w0_t[:], w0cat[:])
            b0_t = cp.tile([P, RW], f32)
            nc.sync.dma_start(b0_t[:], b0cat[:])
            w1_t = cp.tile([P, RW], bf16)
            nc.sync.dma_start(w1_t[:], w1cat[:])
            cb_t = cp.tile([P, L, D], f32)
            nc.sync.dma_start(cb_t[:], convb_rep[:].transpose([1, 0, 2]))
            ow_t = cp.tile([P, NUM_CLASSES], f32)
            nc.sync.dma_start(ow_t[:], out_w[:])
            ob_t = cp.tile([G, NUM_CLASSES], f32)
            nc.sync.dma_start(ob_t[:], outb_rep[:])
            ic_t = cp.tile([G, 1], f32)
            nc.sync.dma_start(ic_t[:], invcnt[:])
            iota3_t = cp.tile([P, P, NBMAX], bf16)
            nc.sync.dma_start(iota3_t[:],
                              iota3_f[:].rearrange("p (s m) -> p s m", s=P))
            dcol_t = cp.tile([P, NBLK], bf16)
            nc.sync.dma_start(dcol_t[:], dcol_in[:])
            idx_t = []
            for q in range(QN):
                it = cp.tile([P, max(NQB[q] * 8, 1)], i16, tag=f"idx{q}")
                nc.sync.dma_start(it[:], idx_in[q][:])
                idx_t.append(it)
            gind_t = cp.tile([P, T, G], bf16)
            nc.sync.dma_start(gind_t[:],
                              gind[:].rearrange("p (t g) -> p t g", t=T))
            xo_t = cp.tile([P, NPC], bf16)
            nc.sync.dma_start(xo_t[:], xT_own[:])

            adl_bf = cp.tile([P, T, H], bf16)
            h_fin = cp.tile([P, T, D], bf16)

            # ---- own a_dst for layer 0 (from own x columns) ----
            for t in range(T):
                ad_ps = pss.tile([P, H], f32, space="PSUM", tag="small")
                nc.tensor.matmul(out=ad_ps[:],
                                 lhsT=xo_t[:, t * P:(t + 1) * P],
                                 rhs=w0_t[:, D + H:D + 2 * H],
                                 start=True, stop=True)
                nc.vector.tensor_tensor(out=adl_bf[:, t, :], in0=ad_ps[:],
                                        in1=b0_t[:, D + H:D + 2 * H],
                                        op=mybir.AluOpType.add)

            # ---- replicated layer-0 node pass: full record table locally ----
            r_all0 = dr.tile([NC * NPC, REC], bf16, tag="r_all0")
            r_all0_rows = r_all0[:].rearrange("(t p) r -> p t r", p=P)
            SUP = 8
            for s in range(GT // SUP):
                xts = sb.tile([P, SUP * P], bf16, tag="xts")
                nc.sync.dma_start(xts[:],
                                  xT_all[:, s * SUP * P:(s + 1) * SUP * P])
                slab = sb.tile([P, SUP, REC], bf16, tag="r0slab")
                for j in range(SUP):
                    w_ps = ps.tile([P, RW], f32, space="PSUM", tag="big")
                    nc.tensor.matmul(out=w_ps[:],
                                     lhsT=xts[:, j * P:(j + 1) * P],
                                     rhs=w0_t[:], start=True, stop=True)
                    nc.vector.tensor_tensor(out=slab[:, j, 0:RW],
                                            in0=w_ps[:], in1=b0_t[:],
                                            op=mybir.AluOpType.add)
                nc.sync.dma_start(
                    r_all0_rows[:, s * SUP:(s + 1) * SUP, :], slab[:])

            r_own1 = dr.tile([NPC, REC], bf16, tag="r_own1")
            r_own1_rows = r_own1[:].rearrange("(t p) r -> p t r", p=P)
            r_all1 = dr.tile([NC * NPC, REC], bf16, tag="r_all1",
                             addr_space="Shared")

            for layer in range(L):
                r_all = r_all0 if layer == 0 else r_all1
                # ---- edge phase ----
                q_off = [0] * QN     # per-class block offset (global)
                ad_off = 0           # all-class block offset (global)
                for (ctiles, kq, nbt_c) in chunks:
                    gq = [None] * QN
                    for q in range(QN):
                        if kq[q] == 0:
                            continue
                        gtile = gp.tile([P, kq[q], REC], bf16, tag=f"g{q}")
                        gq[q] = gtile
                        nc.gpsimd.dma_gather(
                            out_ap=gq[q][:],
                            in_ap=r_all[q * CW:(q + 1) * CW, :],
                            idxs_ap=idx_t[q][:, q_off[q] * 8:(q_off[q] + kq[q]) * 8],
                            num_idxs=kq[q] * P, num_idxs_reg=kq[q] * P,
                            elem_size=REC, single_packet=False)
                    itstr = gp.tile([P, nbt_c, P], bf16, tag="itstr")
                    nc.sync.dma_start(
                        itstr[:], indT_s[:, ad_off * P:(ad_off + nbt_c) * P]
                        .rearrange("p (b s) -> p b s", s=P))
                    # on-chip Ind for the whole chunk ([edge, seg, blk] layout
                    # keeps the packed last dim -> DVE 2x mode)
                    istr = gp.tile([P, P, nbt_c], bf16, tag="istr")
                    nc.vector.tensor_tensor(
                        out=istr[:],
                        in0=dcol_t[:, ad_off:ad_off + nbt_c].unsqueeze(1)
                            .broadcast_to([P, P, nbt_c]),
                        in1=iota3_t[:, :, 0:nbt_c],
                        op=mybir.AluOpType.is_equal)

                    ntc = len(ctiles)
                    S_ps = ps.tile([P, ntc, D], f32, space="PSUM", tag="big")
                    D_ps = pss.tile([P, ntc, H], f32, space="PSUM",
                                    tag="small")
                    qi = [0] * QN    # per-class cursor within gq tiles
                    ci = 0           # block cursor within chunk
                    for tt, t in enumerate(ctiles):
                        nb = int(NBT[t])

                        # a_dst expansion via IndT matmuls into one psum
                        adb_ps = psa.tile([P, nb * H], f32, space="PSUM",
                                          tag="adb")
                        for j in range(nb):
                            nc.tensor.matmul(
                                out=adb_ps[:, j * H:(j + 1) * H],
                                lhsT=itstr[:, ci + j, :], rhs=adl_bf[:, t, :],
                                start=True, stop=True)

                        # attention logits -> leaky -> exp (no CH expansion)
                        att_t = sb.tile([P, nb * H], f32, tag="att_t")
                        a0 = 0
                        for q in range(QN):
                            k = int(BQ[t, q])
                            if k == 0:
                                continue
                            nc.vector.tensor_tensor(
                                out=att_t[:, a0 * H:(a0 + k) * H]
                                    .rearrange("p (k h) -> p k h", k=k),
                                in0=gq[q][:, qi[q]:qi[q] + k, D:D + H],
                                in1=adb_ps[:, a0 * H:(a0 + k) * H]
                                    .rearrange("p (k h) -> p k h", k=k),
                                op=mybir.AluOpType.add)
                            a0 += k
                        lk_t = sb.tile([P, nb * H], f32, tag="lk_t")
                        nc.vector.scalar_tensor_tensor(
                            out=lk_t[:], in0=att_t[:], scalar=NEG_SLOPE,
                            in1=att_t[:], op0=mybir.AluOpType.mult,
                            op1=mybir.AluOpType.max)
                        ex_t = sb.tile([P, nb * H], bf16, tag="ex_t")
                        nc.scalar.activation(
                            out=ex_t[:], in_=lk_t[:],
                            func=mybir.ActivationFunctionType.Exp)

                        # messages (one batched multiply per class run;
                        # channel-major feature layout -> packed last dim)
                        j = 0
                        a0 = 0
                        for q in range(QN):
                            k = int(BQ[t, q])
                            if k == 0:
                                continue
                            msg = sb.tile([P, k * P], bf16, tag="msg")
                            nc.vector.tensor_tensor(
                                out=msg[:].rearrange(
                                    "p (k c h) -> p k c h", k=k, c=CH),
                                in0=gq[q][:, qi[q]:qi[q] + k, 0:D]
                                    .rearrange("p k (c h) -> p k c h", c=CH),
                                in1=ex_t[:, a0 * H:(a0 + k) * H]
                                    .rearrange("p (k h) -> p k h", k=k)
                                    .unsqueeze(2)
                                    .broadcast_to([P, k, CH, H]),
                                op=mybir.AluOpType.mult)
                            for b in range(k):
                                first = (j == 0)
                                last = (j == nb - 1)
                                nc.tensor.matmul(
                                    out=S_ps[:, tt, :],
                                    lhsT=istr[:, :, ci + j],
                                    rhs=msg[:, b * P:(b + 1) * P],
                                    start=first, stop=last)
                                nc.tensor.matmul(
                                    out=D_ps[:, tt, :],
                                    lhsT=istr[:, :, ci + j],
                                    rhs=ex_t[:, j * H:(j + 1) * H],
                                    start=first, stop=last)
                                j += 1
                            qi[q] += k
                            a0 += k
                        ci += nb

                    # paired epilogue over the chunk's tiles:
                    # divide, bias, ELU
                    t0 = ctiles[0]
                    d8 = sb.tile([P, ntc * H], f32, tag="d8")
                    nc.vector.tensor_scalar(
                        out=d8[:], in0=D_ps[:], scalar1=1e-12,
                        scalar2=None, op0=mybir.AluOpType.max)
                    rc = sb.tile([P, ntc * H], f32, tag="rc")
                    nc.vector.reciprocal(rc[:], d8[:])
                    hr = sb.tile([P, ntc * D], f32, tag="hr")
                    nc.vector.tensor_tensor(
                        out=hr[:].rearrange("p (t c h) -> p t c h",
                                            t=ntc, c=CH),
                        in0=S_ps[:].rearrange("p t (c h) -> p t c h", c=CH),
                        in1=rc[:].rearrange("p (t h) -> p t h", t=ntc)
                            .unsqueeze(2).broadcast_to([P, ntc, CH, H]),
                        op=mybir.AluOpType.mult)
                    h1 = sb.tile([P, ntc * D], f32, tag="h1")
                    nc.vector.tensor_tensor(
                        out=h1[:].rearrange("p (t d) -> p t d", t=ntc),
                        in0=hr[:].rearrange("p (t d) -> p t d", t=ntc),
                        in1=cb_t[:, layer, :].unsqueeze(1)
                            .broadcast_to([P, ntc, D]),
                        op=mybir.AluOpType.add)
                    ng = sb.tile([P, ntc * D], f32, tag="ng")
                    nc.vector.tensor_scalar(
                        out=ng[:], in0=h1[:], scalar1=0.0, scalar2=None,
                        op0=mybir.AluOpType.min)
                    pz = sb.tile([P, ntc * D], f32, tag="pz")
                    nc.scalar.activation(
                        out=pz[:], in_=h1[:],
                        func=mybir.ActivationFunctionType.Relu)
                    em = sb.tile([P, ntc * D], f32, tag="em")
                    nc.scalar.activation(
                        out=em[:], in_=ng[:],
                        func=mybir.ActivationFunctionType.Exp)
                    if layer == 0:
                        # ELU -> bf16, then fused layer-1 node phase
                        rec1_sb = sb.tile([P, ntc, REC], bf16, tag="rec1")
                        hb = sb.tile([P, ntc, D], bf16, tag="hb")
                        nc.vector.scalar_tensor_tensor(
                            out=hb[:].rearrange("p t d -> p (t d)"),
                            in0=em[:], scalar=-1.0,
                            in1=pz[:], op0=mybir.AluOpType.add,
                            op1=mybir.AluOpType.add)
                        for tt, t in enumerate(ctiles):
                            hT_ps = ps.tile([P, P], bf16, space="PSUM",
                                            tag="big")
                            nc.tensor.transpose(out=hT_ps[:],
                                                in_=hb[:, tt, :],
                                                identity=ident[:])
                            hTb = sb.tile([P, P], bf16, tag="hTb")
                            nc.vector.tensor_copy(hTb[:], hT_ps[:])
                            w_ps = ps.tile([P, RW], f32, space="PSUM",
                                           tag="big")
                            nc.tensor.matmul(out=w_ps[:], lhsT=hTb[:],
                                             rhs=w1_t[:], start=True,
                                             stop=True)
                            nc.vector.tensor_copy(rec1_sb[:, tt, 0:RW],
                                                  w_ps[:])
                            nc.vector.tensor_copy(adl_bf[:, t, :],
                                                  w_ps[:, D + H:D + 2 * H])
                        nc.sync.dma_start(
                            r_own1_rows[:, t0:t0 + ntc, :], rec1_sb[:])
                    else:
                        nc.vector.scalar_tensor_tensor(
                            out=h_fin[:, t0:t0 + ntc, :]
                                .rearrange("p t d -> p (t d)"),
                            in0=em[:], scalar=-1.0,
                            in1=pz[:], op0=mybir.AluOpType.add,
                            op1=mybir.AluOpType.add)
                    for q in range(QN):
                        q_off[q] += kq[q]
                    ad_off += nbt_c

                if layer == 0:
                    nc.gpsimd.collective_compute(
                        "AllGather", mybir.AluOpType.bypass,
                        replica_groups=[list(range(NC))],
                        ins=[r_own1.opt()], outs=[r_all1.opt()],
                    )

            # ---- readout ----
            pool_ps = ps.tile([P, G], f32, space="PSUM", tag="big")
            for t in range(T):
                nc.tensor.matmul(out=pool_ps[:], lhsT=h_fin[:, t, :],
                                 rhs=gind_t[:, t, :],
                                 start=(t == 0), stop=(t == T - 1))
            pool_sb = sb.tile([P, G], f32, tag="pool_sb")
            nc.vector.tensor_copy(pool_sb[:], pool_ps[:])
            ar_in = dr.tile([P, G], f32, tag="ar_in")
            nc.sync.dma_start(ar_in[:], pool_sb[:])
            ar_out = dr.tile([NC * P, G], f32, tag="ar_out",
                             addr_space="Shared")
            nc.gpsimd.collective_compute(
                "AllGather", mybir.AluOpType.bypass,
                replica_groups=[list(range(NC))],
                ins=[ar_in.opt()], outs=[ar_out.opt()],
            )
            ars = sb.tile([P, NC, G], f32, tag="ars")
            nc.sync.dma_start(ars[:],
                              ar_out[:].rearrange("(c p) g -> p c g", p=P))
            s4 = sb.tile([P, 4, G], f32, tag="s4")
            nc.vector.tensor_tensor(out=s4[:], in0=ars[:, 0:4, :],
                                    in1=ars[:, 4:8, :],
                                    op=mybir.AluOpType.add)
            s2 = sb.tile([P, 2, G], f32, tag="s2")
            nc.vector.tensor_tensor(out=s2[:], in0=s4[:, 0:2, :],
                                    in1=s4[:, 2:4, :],
                                    op=mybir.AluOpType.add)
            s1 = sb.tile([P, G], f32, tag="s1")
            nc.vector.tensor_tensor(out=s1[:], in0=s2[:, 0, :],
                                    in1=s2[:, 1, :],
                                    op=mybir.AluOpType.add)
            log_ps = pss.tile([G, NUM_CLASSES], f32, space="PSUM",
                              tag="small")
            nc.tensor.matmul(out=log_ps[:], lhsT=s1[:], rhs=ow_t[:],
                             start=True, stop=True)
            sc = sb.tile([G, NUM_CLASSES], f32, tag="sc")
            nc.vector.tensor_scalar(
                out=sc[:], in0=log_ps[:], scalar1=ic_t[:, :1], scalar2=None,
                op0=mybir.AluOpType.mult)
            ofin = sb.tile([G, NUM_CLASSES], f32, tag="ofin")
            nc.vector.tensor_tensor(out=ofin[:], in0=sc[:], in1=ob_t[:],
                                    op=mybir.AluOpType.add)
            nc.sync.dma_start(out[:], ofin[:])

    nc.finalize()
    return nc


def kernel(x, edge_index, batch, in_w, in_b, conv_w, conv_b, att_src, att_dst,
           out_w, out_b):
    x = np.asarray(x, dtype=np.float32)
    edge_index = np.asarray(edge_index)
    batch = np.asarray(batch)
    in_w = np.asarray(in_w, dtype=np.float32)
    in_b = np.asarray(in_b, dtype=np.float32)
    conv_w = np.asarray(conv_w, dtype=np.float32)
    conv_b = np.asarray(conv_b, dtype=np.float32)
    att_src = np.asarray(att_src, dtype=np.float32)
    att_dst = np.asarray(att_dst, dtype=np.float32)
    out_w = np.asarray(out_w, dtype=np.float32)
    out_b = np.asarray(out_b, dtype=np.float32)

    N, F_IN = x.shape
    E = edge_index.shape[1]
    L, D, _ = conv_w.shape
    G = int(batch.max()) + 1
    NUM_CLASSES = out_w.shape[1]

    key = (N, E, F_IN, D, G, L,
           hash(edge_index.tobytes()), hash(batch.tobytes()))
    if key not in _PLAN_CACHE:
        plan = _plan(N, E, D, G, L, edge_index, batch)
        nc = _build(plan, D, G, L, NUM_CLASSES)
        _PLAN_CACHE[key] = (plan, nc)
    plan, nc = _PLAN_CACHE[key]

    NPCR, NPC, T = plan["NPCR"], plan["NPC"], plan["T"]

    A_src = np.zeros((L, D, H), dtype=np.float32)
    A_dst = np.zeros((L, D, H), dtype=np.float32)
    for l in range(L):
        for h in range(H):
            A_src[l, h * CH:(h + 1) * CH, h] = att_src[l, h]
            A_dst[l, h * CH:(h + 1) * CH, h] = att_dst[l, h]

    # channel-major feature permutation: stored col c*H+h = standard h*CH+c
    permP = np.empty(D, dtype=np.int64)
    for h in range(H):
        for c in range(CH):
            permP[c * H + h] = h * CH + c

    w0 = in_w @ conv_w[0]
    w0cat = np.concatenate([w0[:, permP], w0 @ A_src[0], w0 @ A_dst[0]],
                           axis=1).astype(ml_dtypes.bfloat16)
    b0 = in_b @ conv_w[0]
    b0cat_rep = np.tile(
        np.concatenate([b0[permP], b0 @ A_src[0], b0 @ A_dst[0]]),
        (P, 1)).astype(np.float32)
    w1cat = np.concatenate(
        [conv_w[1][permP][:, permP], (conv_w[1] @ A_src[1])[permP],
         (conv_w[1] @ A_dst[1])[permP]],
        axis=1).astype(ml_dtypes.bfloat16)
    convb_rep = np.tile(conv_b[:, None, permP],
                        (1, P, 1)).astype(np.float32)
    outb_rep = np.tile(out_b, (G, 1)).astype(np.float32)
    out_w_p = np.ascontiguousarray(out_w[permP, :])

    xs = np.zeros((NC, NPC, F_IN), dtype=np.float32)
    xs[:, :NPCR] = x[:NC * NPCR].reshape(NC, NPCR, F_IN)
    xT_all = np.ascontiguousarray(
        xs.reshape(NC * NPC, F_IN).T).astype(ml_dtypes.bfloat16)

    NBMAX = max(ch[2] for ch in plan["chunks"])
    iota3 = np.broadcast_to(
        np.arange(P, dtype=np.float32)[None, :, None],
        (P, P, NBMAX)).reshape(P, P * NBMAX).astype(ml_dtypes.bfloat16)

    in_maps = []
    for c in range(NC):
        pc = plan["per_core"][c]
        m = dict(
            xT_all=xT_all,
            xT_own=np.ascontiguousarray(
                xT_all[:, c * NPC:(c + 1) * NPC]),
            w0cat=w0cat, b0cat=b0cat_rep, w1cat=w1cat,
            convb_rep=convb_rep, out_w=out_w_p, outb_rep=outb_rep,
            invcnt=plan["invcnt"], iota3_f=np.ascontiguousarray(iota3),
            dcol=pc["dcol"].astype(ml_dtypes.bfloat16),
            indT_s=pc["indT_s"],
            gind=np.ascontiguousarray(
                pc["gind"].reshape(P, T * G)),
        )
        for q in range(QN):
            m[f"idx{q}"] = pc["idx"][q]
        in_maps.append(m)

    global LAST_EXEC_NS, LAST_RESULTS
    res = run_bass_kernel_spmd(nc, in_maps, list(range(NC)), trace=TRACE)
    LAST_EXEC_NS = res.exec_time_ns
    LAST_RESULTS = res
    return np.asarray(res.results[0]["out"], dtype=np.float32)
